# revision 16
# baseline (speedup 1.0000x reference)
"""AGCA channel-attention forward, data-parallel across 8 TRN2 NeuronCores.

Reference computation (per batch element b):
    y[b,c]   = mean(x[b,c,:,:])                      # global avg pool
    y1[b,h]  = sum_c y[b,c] * W1[h,c]                # 1x1 conv == matmul
    a[b,:]   = softmax(w2 * y1[b,:])                 # over hidden dim
    z[b,k]   = y1[b,k]*a[b,k] + sum_h y1[b,h]*A2[h,k]
    zr       = relu(w3 * z)
    g[b,c]   = sigmoid(sum_h zr[b,h] * W4[c,h])
    out      = x * g[:, :, None, None]

Sharding: pure data parallel on batch (32 -> 4 per core); the tiny params
are replicated. No collectives.

The kernel is HBM-stream-bound: every byte of x must be read and every
byte of out written, and the trace shows the DMA stream gapless at the
16-SDMA-engine line rate, with a fixed ~8.4 us runtime pre/postamble
around it. The one real lever is bytes, so x is cast to fp16 on the host
(the module's output tolerance is rel-L2 < 2e-2; fp16 rounding of x and
of the product contributes ~2e-3) and the product is stored as fp16 and
upcast on the host. That halves the stream: 6.42 MB in + 6.42 MB out per
core.

Host-side folding (all inside kernel(), which receives the raw inputs):
  - x is pre-transposed to [128, KBLK*HW] so every partition's whole
    shard is contiguous in DRAM: every load/store DMA is a flat 2D copy
    with uniform 12544-byte per-partition descriptors (the AP shape the
    trace shows running fastest; the f32 kernel's interleaved/chunked
    load APs ran ~7% slower than its stores).
  - W1/W4 are pre-transposed to the layouts the TensorEngine wants.
  - the 1/(H*W) of the mean and s3 = sign(w3) (s3 := 1 when w3 == 0) fold
    into W1TS, pushed through the linear ops so relu(w3*z) =
    |w3| * relu(sign(w3)*z); |w3| folds into W4. The softmax pre-scale is
    recovered from the s3-scaled y1 with the single scalar w2*s3 (s3^2=1),
    applied as the Exp activation's scale. No max-subtraction: the pooled
    projections are O(0.1), far from exp overflow, and softmax is
    shift-invariant in exact math.
  - all params pack into ONE [128, 450] f32 tensor -> one DMA, one copy.
  - sigmoid is evaluated as 1/(1+exp(-v)) so the Scalar engine only ever
    uses the Exp activation table (no per-batch table reloads).

Per-core dataflow: one load DMA and one store DMA per batch (1.6 MB each,
all on the Sync HWDGE ring, whose FIFO drains every load before the first
store so writes never steal bandwidth from reads later batches' chains
depend on). Per batch: spatial row-sums (block hf=0 on DVE reduce, block
hf=1 on ACT identity-copy with free-dim accumulator, in parallel, both
accumulating in f32), the tiny per-batch MLP on PE/ACT/DVE, then the
in-place per-partition-scalar gate multiplies split DVE/ACT and the
batch's store right behind them. Explicit ordering deps pin each next
batch's sums ahead of the current batch's big multiplies in the DVE/ACT
instruction streams. The fp16 shard stays resident in SBUF (6.4 MB).
"""

import numpy as np

import concourse.bacc as bacc
import concourse.bass as bass
import concourse.mybir as mybir
import concourse.tile as tile
from concourse.bass_utils import run_bass_kernel_spmd

# Problem shapes (hardcoded: kernel.py must be self-contained).
B, C, H, W = 32, 256, 56, 56
HIDE = 64
NCORES = 8
BL = B // NCORES  # batches per core = 4
HW = H * W  # 3136
ROWS = BL * C  # 1024 rows per core
KBLK = ROWS // 128  # 8 blocks of 128 rows
F32 = mybir.dt.float32
F16 = mybir.dt.float16
AX = mybir.AxisListType
AF = mybir.ActivationFunctionType
OP = mybir.AluOpType

# Packed-parameter column layout: [s3*W1T | A2 | |w3|*W4T | 1.0 | w2*s3]
PCOLS_W1S = 0  # [128, 2*HIDE]
PCOLS_A2 = 2 * HIDE  # [64, HIDE]
PCOLS_W4 = 3 * HIDE  # [64, C]
PCOLS_ONE = 3 * HIDE + C  # [1, 1] == 1.0 (transpose identity)
PCOLS_W2S = PCOLS_ONE + 1  # [1, 1] == w2*s3
PCOLS = PCOLS_W2S + 1  # 450


def _build() -> bass.Bass:
    nc = bacc.Bacc("TRN2", target_bir_lowering=False)
    x_d = nc.dram_tensor("x", [128, KBLK * HW], F16, kind="ExternalInput")
    params_d = nc.dram_tensor("PARAMS", [128, PCOLS], F32, kind="ExternalInput")
    out_d = nc.dram_tensor("out", [128, KBLK * HW], F16, kind="ExternalOutput")

    with tile.TileContext(nc) as tc:
        with (
            tc.tile_pool(name="big", bufs=1) as big,
            tc.tile_pool(name="consts", bufs=1) as consts,
            tc.tile_pool(name="small", bufs=2) as small,
            tc.tile_pool(name="gpool", bufs=1) as gpool,
            tc.tile_pool(name="psm1", bufs=1, space="PSUM") as psm1,
            tc.tile_pool(name="psm2", bufs=2, space="PSUM") as psm2,
            tc.tile_pool(name="psg", bufs=2, space="PSUM") as psg,
        ):
            # ---- params: one DMA + one DVE funnel copy. The DMA goes FIRST
            # on the Sync ring, ahead of the x loads: 230 KB costs the x
            # stream ~0.6 us, but params land by ~8.5 us so the DVE funnel
            # copy is long done before the first batch's sums need the
            # engine. (On the ACT ring it drains at ~50 GB/s round-robining
            # against the saturated Sync ring and stalls DVE ~3 us; on the
            # GpSimd SWDGE path it lands even later.)
            p_raw = consts.tile([128, PCOLS], F32)
            nc.sync.dma_start(out=p_raw[:, :], in_=params_d[:, :])
            ps = consts.tile([128, PCOLS], F32)
            nc.vector.tensor_copy(out=ps[:, :], in_=p_raw[:, :])

            w1s = ps[:, PCOLS_W1S:PCOLS_A2].rearrange(
                "p (h d) -> p h d", h=2
            )  # [128, 2, HIDE]
            a2s = ps[:HIDE, PCOLS_A2:PCOLS_W4]  # [64, 64]
            w4ts = ps[:HIDE, PCOLS_W4:PCOLS_ONE]  # [64, 256]
            i1 = ps[:1, PCOLS_ONE : PCOLS_ONE + 1]  # [1, 1] == 1.0
            w2s = ps[:1, PCOLS_W2S : PCOLS_W2S + 1]  # [1, 1] == w2*s3

            xt = big.tile([128, KBLK * HW], F16)
            ysum = gpool.tile([128, BL, 2], F32)  # ysum[p, b, hf] = row sum
            gt = gpool.tile([128, BL, 2], F32)  # gt[p, b, hf] gates blk 2b+hf
            s_all = gpool.tile([1, BL], F32)  # softmax denominators

            def blk(k):
                return xt[:, k * HW : (k + 1) * HW]

            # one flat contiguous load per batch, all queued upfront on the
            # Sync HWDGE ring (no waits): uniform [128 x 12544B] descriptors
            # (per-block 6272B-packet loads measured ~12% slower).
            for b in range(BL):
                nc.sync.dma_start(
                    out=xt[:, 2 * b * HW : (2 * b + 2) * HW],
                    in_=x_d[:, 2 * b * HW : (2 * b + 2) * HW],
                )

            HWH = HW // 2
            HWQ = HW // 4

            def emit_sums(b, prev_exp=None):
                """Per-row spatial sums for one batch. Measured DVE rates:
                TENSOR_REDUCE is ~1.09 ns/col regardless of dtype, but
                fp16+fp16->fp16 tensor_tensor adds run at ~0.62 ns/col --
                so block hf=0 is summed as a 2-level fp16 halving tree
                (0.97 + 0.49 us) plus a quarter-length reduce (0.89 us),
                2.35 us total vs 3.4 us for the plain reduce. Block hf=1
                goes to ACT as an in-place identity copy with the free-dim
                accumulator (2.9 us), which also leaves DVE the headroom to
                run BOTH gate multiplies (fp16 tensor_scalar is the fastest
                block op at ~0.31 ns/col). Returns the last DVE instruction
                for ordering pins."""
                blkv = blk(2 * b)
                t1 = small.tile([128, HWH], F16, tag="sumt1")
                nc.vector.tensor_add(
                    out=t1[:, :], in0=blkv[:, 0:HWH], in1=blkv[:, HWH:HW]
                )
                t2 = small.tile([128, HWQ], F16, tag="sumt2")
                nc.vector.tensor_add(
                    out=t2[:, :], in0=t1[:, 0:HWQ], in1=t1[:, HWQ:HWH]
                )
                last = nc.vector.reduce_sum(
                    out=ysum[:, b, 0:1], in_=t2[:, :], axis=AX.X
                )
                act_sum = nc.scalar.activation(
                    out=blk(2 * b + 1),
                    in_=blk(2 * b + 1),
                    func=AF.Copy,
                    accum_out=ysum[:, b, 1:2],
                )
                if prev_exp is not None:
                    # Keep ACT's stream in pipeline order: the previous
                    # batch's tiny sigmoid-exp must run before this 2.9 us
                    # accum, or the gate (and so the store) slips behind it.
                    tile.add_dep_helper(
                        act_sum.ins, prev_exp.ins, sync=False,
                        reason="order prev-batch exps before next ACT accum",
                    )
                return last

            def emit_mlp(b):
                """ysum[:, b, :] -> gate column gt[:, b, :] for one batch."""

                # y1s = s3 * y @ W1^T in both orientations via swapped roles
                y1p = psm2.tile([1, HIDE], F32, tag="y1")
                y1tp = psm1.tile([HIDE, 1], F32, tag="y1t")
                for h in range(2):
                    nc.tensor.matmul(
                        y1p[:, :], ysum[:, b, h : h + 1], w1s[:, h, :],
                        start=(h == 0), stop=(h == 1),
                    )
                for h in range(2):
                    nc.tensor.matmul(
                        y1tp[:, :], w1s[:, h, :], ysum[:, b, h : h + 1],
                        start=(h == 0), stop=(h == 1),
                    )
                y1ts = small.tile([HIDE, 1], F32, tag="y1ts")
                nc.vector.tensor_copy(out=y1ts[:, :], in_=y1tp[:, :])

                # a = softmax((w2*s3) * y1s) over hid; exp straight off PSUM
                e = small.tile([1, HIDE], F32, tag="e")
                nc.scalar.activation(
                    out=e[:, :], in_=y1p[:, :], func=AF.Exp,
                    scale=w2s, accum_out=s_all[:, b : b + 1],
                )
                r = small.tile([1, 1], F32, tag="r")
                nc.vector.reciprocal(out=r[:, :], in_=s_all[:, b : b + 1])
                a = small.tile([1, HIDE], F32, tag="a")
                nc.vector.tensor_scalar_mul(out=a[:, :], in0=e[:, :], scalar1=r[:, :])

                # zT' = s3*y1T * aT + A2^T @ (s3*y1T);  zr = relu(zT')
                p3 = psm1.tile([HIDE, 1], F32, tag="p3")
                nc.tensor.matmul(p3[:, :], a2s, y1ts[:, :], start=True, stop=True)
                atp = psm1.tile([HIDE, 1], F32, tag="at")
                nc.tensor.transpose(atp[:, :], a[:, :], i1)
                zt = small.tile([HIDE, 1], F32, tag="zt")
                nc.vector.tensor_mul(out=zt[:, :], in0=y1ts[:, :], in1=atp[:, :])
                zr = small.tile([HIDE, 1], F32, tag="zr")
                nc.vector.tensor_scalar(
                    out=zr[:, :], in0=zt[:, :],
                    scalar1=p3[:, 0:1], scalar2=0.0,
                    op0=OP.add, op1=OP.max,
                )

                # g = sigmoid(v) = 1/(1 + exp(-v)), v = |w3| * W4 @ zr per
                # channel half; both halves share one [128, 2] pipeline:
                # exp on ACT (Exp table stays loaded), add+reciprocal on DVE.
                gp = psg.tile([128, 2], F32, tag="g")
                for hf in range(2):
                    nc.tensor.matmul(
                        gp[:, hf : hf + 1],
                        w4ts[:, hf * 128 : (hf + 1) * 128], zr[:, :],
                        start=True, stop=True,
                    )
                ge = small.tile([128, 2], F32, tag="ge")
                g_exp = nc.scalar.activation(
                    out=ge[:, :], in_=gp[:, :], func=AF.Exp, scale=-1.0
                )
                gd = small.tile([128, 2], F32, tag="gd")
                nc.vector.tensor_scalar_add(out=gd[:, :], in0=ge[:, :], scalar1=1.0)
                nc.vector.reciprocal(out=gt[:, b, 0:2], in_=gd[:, :])
                return g_exp

            def emit_gate_store(b, next_sums=None):
                """In-place gate multiplies -- BOTH blocks on DVE, whose
                fp16 tensor_scalar is ~3x faster than an ACT block op --
                + one store for the whole batch right behind them.
                next_sums pins the next batch's row-sums ahead of the
                multiplies in DVE's in-order stream so the tail batch's
                chain starts as soon as its load lands."""
                for hf in range(2):
                    dve_mul = nc.vector.tensor_scalar_mul(
                        out=blk(2 * b + hf),
                        in0=blk(2 * b + hf),
                        scalar1=gt[:, b, hf : hf + 1],
                    )
                    if next_sums is not None:
                        tile.add_dep_helper(
                            dve_mul.ins, next_sums.ins, sync=False,
                            reason="order next-batch DVE sums before big mul",
                        )
                # same Sync ring as the loads: ring FIFO drains every load
                # descriptor before the first store, so writes never steal
                # bandwidth from reads that later batches' chains depend on.
                nc.sync.dma_start(
                    out=out_d[:, 2 * b * HW : (2 * b + 2) * HW],
                    in_=xt[:, 2 * b * HW : (2 * b + 2) * HW],
                )

            emit_sums(0)
            for b in range(BL):
                g_exp = emit_mlp(b)
                next_sums = (
                    emit_sums(b + 1, prev_exp=g_exp) if b + 1 < BL else None
                )
                emit_gate_store(b, next_sums)

    nc.compile()
    return nc


_CACHE: dict = {}


def _get_nc() -> bass.Bass:
    if "nc" not in _CACHE:
        _CACHE["nc"] = _build()
    return _CACHE["nc"]


def _prep_params(inputs: dict) -> np.ndarray:
    W1 = np.asarray(inputs["W1"], dtype=np.float32)
    W4 = np.asarray(inputs["W4"], dtype=np.float32)
    w2 = float(np.asarray(inputs["w2"], dtype=np.float32)[0])
    w3 = float(np.asarray(inputs["w3"], dtype=np.float32)[0])
    A2 = np.asarray(inputs["A2"], dtype=np.float32)
    assert W1.shape == (HIDE, C) and W4.shape == (C, HIDE)

    # [p, h, hid] layout: W1T[h*128+p, hid] with the channel half h as the
    # middle axis so both halves sit in one contiguous column block.
    base = (W1 / HW).T.reshape(2, 128, HIDE).transpose(1, 0, 2)  # [128, 2, HIDE]
    s3 = 1.0 if w3 == 0.0 else float(np.sign(w3))

    params = np.zeros((128, PCOLS), dtype=np.float32)
    params[:, PCOLS_W1S:PCOLS_A2] = (s3 * base).reshape(128, 2 * HIDE)
    params[:HIDE, PCOLS_A2:PCOLS_W4] = A2
    params[:HIDE, PCOLS_W4:PCOLS_ONE] = abs(w3) * W4.T
    params[0, PCOLS_ONE] = 1.0
    params[0, PCOLS_W2S] = w2 * s3
    return params


def _run(inputs: dict, trace: bool = False):
    x = np.asarray(inputs["x"], dtype=np.float32)
    assert x.shape == (B, C, H, W)
    params = _prep_params(inputs)

    # Row i = b*C + c of a shard lives at partition i % 128, block i // 128;
    # the device layout [p, k*HW] keeps each partition's 8 blocks contiguous.
    rows = x.reshape(NCORES, KBLK, 128, HW).transpose(0, 2, 1, 3)  # [n, p, k, c]
    xf16 = np.ascontiguousarray(rows.reshape(NCORES, 128, KBLK * HW)).astype(
        np.float16
    )

    in_maps = [{"x": xf16[i], "PARAMS": params} for i in range(NCORES)]

    res = run_bass_kernel_spmd(
        _get_nc(), in_maps, core_ids=list(range(NCORES)), trace=trace
    )
    outs = [
        r["out"]
        .reshape(128, KBLK, HW)
        .transpose(1, 0, 2)
        .astype(np.float32)
        .reshape(BL, C, H, W)
        for r in res.results
    ]
    return np.concatenate(outs, axis=0), res


def kernel(**inputs) -> np.ndarray:
    out, _ = _run(inputs)
    return out
